# revision 3
# baseline (speedup 1.0000x reference)
"""Quantized 3x3 conv (int8-style QAT conv) on 8 TRN2 NeuronCores.

Reference semantics:
    qx = clip(round(x * (127/3)), -127, 127)          # int values in f32
    qw = clip(round(w * (127/0.05)), -127, 127)
    out = conv2d(qx, qw, stride 1, pad 1) * (3*0.05/127^2) + bias[None,:,None,None]

Strategy: pure data parallelism over batch (32 images -> 4 per core).
Each core:
  - DMAs its 4 images in, quantizes on-chip (magic-number round + clamp)
    into a zero-padded [128ci, 58, 58] bf16 tile per image (ints <= 127 are
    exact in bf16).
  - Weights are DMA'd as [ci, tap, co] (host provides a pure layout permute),
    quantized on-chip to bf16.
  - The 3x3 conv is 9 shifted matmuls accumulated in PSUM: for each output
    row-chunk of 8 rows (N=448 <= 512 PSUM bank) and each 128-wide cout
    chunk, out[co, y, x] += qw[tap][ci, co].T @ qx[ci, y+dy, x+dx].
  - Epilogue fuses rescale + bias on ScalarE from PSUM, DMA out.
"""

import numpy as np

import concourse.bass as bass
import concourse.mybir as mybir
import concourse.tile as tile
from concourse import bacc
from concourse.bass_utils import run_bass_kernel_spmd

# Problem constants
B, CIN, COUT, H, W, KS = 32, 128, 256, 56, 56, 3
NCORES = 8
BPC = B // NCORES          # images per core
NPIX = H * W               # 3136
HP = H + 2                 # padded spatial
QL = 127.0
SX = QL / 3.0              # activation quant scale
SW = QL / 0.05             # weight quant scale
RESCALE = (3.0 * 0.05) / (QL * QL)
MAGIC = 1.5 * 2.0**23      # fp32 round-to-nearest-even trick

ROWS = 8                   # output rows per matmul tile
RT = H // ROWS             # 7 row tiles per image
NTAP = KS * KS
NCHUNK = COUT // 128       # 2 cout chunks
GROUP = 4                  # psum tiles sharing one weight load

F32 = mybir.dt.float32
BF16 = mybir.dt.bfloat16

_NC = None


def _build(reps: int = 1):
    """Build the SPMD graph. reps>1 wraps the whole per-call pipeline in a
    hardware For loop — used only by the timing harness (bench.py) to
    measure per-iteration HW time through the high-latency tunnel."""
    nc = bacc.Bacc("TRN2", target_bir_lowering=False, num_devices=NCORES)

    x_t = nc.dram_tensor("x", [BPC, CIN, NPIX], F32, kind="ExternalInput")
    w_t = nc.dram_tensor("weight", [NTAP, CIN, COUT], F32, kind="ExternalInput")
    b_t = nc.dram_tensor("bias", [NCHUNK, 128, 1], F32, kind="ExternalInput")
    o_t = nc.dram_tensor("out", [BPC, NCHUNK, 128, NPIX], F32, kind="ExternalOutput")

    with tile.TileContext(nc) as tc:
        with (
            tc.tile_pool(name="consts", bufs=1) as consts,
            tc.tile_pool(name="xq", bufs=1) as xqp,
            tc.tile_pool(name="xstage", bufs=2) as xsp,
            tc.tile_pool(name="tmp", bufs=2) as tmpp,
            tc.tile_pool(name="outp", bufs=6) as outp,
            tc.tile_pool(name="psum", bufs=8, space="PSUM") as psp,
        ):
            # ---- weights: DMA [ci, tap, co], quantize to bf16 ----
            wraw = consts.tile([128, NTAP, COUT], F32, tag="wraw")
            nc.sync.dma_start(out=wraw[:], in_=w_t[:].rearrange("t p c -> p t c"))
            wtmp = consts.tile([128, NTAP, COUT], F32, tag="wtmp")
            nc.scalar.activation(
                wtmp[:], wraw[:], mybir.ActivationFunctionType.Copy,
                bias=MAGIC, scale=SW,
            )
            nc.vector.tensor_scalar(
                wtmp[:], wtmp[:], MAGIC, -QL,
                mybir.AluOpType.subtract, mybir.AluOpType.max,
            )
            wq = consts.tile([128, NTAP, COUT], BF16, tag="wq")
            nc.vector.tensor_scalar_min(wq[:], wtmp[:], QL)

            bias_sb = []
            for c in range(NCHUNK):
                bs = consts.tile([128, 1], F32, tag=f"bias{c}")
                nc.sync.dma_start(out=bs[:], in_=b_t[c])
                bias_sb.append(bs)

            # ---- padded quantized activations, one tile per image ----
            xq = []
            for b in range(BPC):
                t = xqp.tile([128, HP, HP], BF16, tag=f"xq{b}")
                nc.vector.memset(t[:], 0.0)
                xq.append(t)

            def body(_iv=None):
                for b in range(BPC):
                    xs = xsp.tile([128, NPIX], F32, tag="xs", name=f"xs{b}")
                    nc.sync.dma_start(out=xs[:], in_=x_t[b])
                    t1 = tmpp.tile([128, NPIX], F32, tag="t1", name=f"t1_{b}")
                    nc.scalar.activation(
                        t1[:], xs[:], mybir.ActivationFunctionType.Copy,
                        bias=MAGIC, scale=SX,
                    )
                    nc.vector.tensor_scalar(
                        t1[:], t1[:], MAGIC, -QL,
                        mybir.AluOpType.subtract, mybir.AluOpType.max,
                    )
                    nc.vector.tensor_scalar_min(
                        xq[b][:, 1 : H + 1, 1 : W + 1],
                        t1[:].rearrange("p (h w) -> p h w", h=H),
                        QL,
                    )

                # ---- conv: 9 shifted matmuls accumulated in PSUM ----
                tiles = [(b, r) for b in range(BPC) for r in range(RT)]
                assert len(tiles) % GROUP == 0
                for g in range(len(tiles) // GROUP):
                    grp = tiles[GROUP * g : GROUP * (g + 1)]
                    for c in range(NCHUNK):
                        pts = [
                            psp.tile([128, ROWS * W], F32, tag="pt",
                                     name=f"pt{g}_{c}_{i}")
                            for i, _ in enumerate(grp)
                        ]
                        for tap in range(NTAP):
                            ky, kx = divmod(tap, KS)
                            lhsT = wq[:, tap, c * 128 : (c + 1) * 128]
                            for t, (b, r) in enumerate(grp):
                                rhs = xq[b][:, r * ROWS + ky : r * ROWS + ky + ROWS,
                                            kx : kx + W]
                                nc.tensor.matmul(
                                    pts[t][:], lhsT, rhs,
                                    start=(tap == 0), stop=(tap == NTAP - 1),
                                )
                        for t, (b, r) in enumerate(grp):
                            ot = outp.tile([128, ROWS * W], F32, tag="ot",
                                           name=f"ot{g}_{c}_{t}")
                            nc.scalar.activation(
                                ot[:], pts[t][:],
                                mybir.ActivationFunctionType.Identity,
                                bias=bias_sb[c][:], scale=RESCALE,
                            )
                            nc.sync.dma_start(
                                out=o_t[b, c, :, r * ROWS * W : (r + 1) * ROWS * W],
                                in_=ot[:],
                            )

            if reps == 1:
                body()
            else:
                with tc.For_i(0, reps, 1):
                    body()
    nc.compile()
    return nc


def _get_nc():
    global _NC
    if _NC is None:
        _NC = _build()
    return _NC


def kernel(x: np.ndarray, weight: np.ndarray, bias: np.ndarray, trace: bool = False):
    """Full inputs in, full output out. Shards batch across 8 cores."""
    x = np.ascontiguousarray(x, dtype=np.float32).reshape(B, CIN, NPIX)
    # pure layout permute: [co, ci, ky, kx] -> [ky*kx, ci, co]
    w_l = np.ascontiguousarray(
        weight.astype(np.float32).transpose(2, 3, 1, 0)
    ).reshape(NTAP, CIN, COUT)
    b_l = np.ascontiguousarray(bias.astype(np.float32)).reshape(NCHUNK, 128, 1)

    nc = _get_nc()
    in_maps = [
        {
            "x": np.ascontiguousarray(x[i * BPC : (i + 1) * BPC]),
            "weight": w_l,
            "bias": b_l,
        }
        for i in range(NCORES)
    ]
    res = run_bass_kernel_spmd(nc, in_maps, core_ids=list(range(NCORES)), trace=trace)
    out = np.concatenate(
        [r["out"].reshape(BPC, COUT, H, W) for r in res.results], axis=0
    )
    if trace:
        kernel.last_results = res
    return out


# revision 5
# speedup vs baseline: 15.0952x; 15.0952x over previous
"""Quantized 3x3 conv (int8-style QAT conv) on 8 TRN2 NeuronCores.

Reference semantics:
    qx = clip(round(x * (127/3)), -127, 127)          # int values in f32
    qw = clip(round(w * (127/0.05)), -127, 127)
    out = conv2d(qx, qw, stride 1, pad 1) * (3*0.05/127^2) + bias[None,:,None,None]

Strategy: pure data parallelism over batch (32 images -> 4 per core).
Each core:
  - DMAs its 4 images in, quantizes on-chip (magic-number round + clamp)
    into a zero-padded [128ci, 58, 58] bf16 tile per image (ints <= 127 are
    exact in bf16).
  - Weights are DMA'd as [ci, tap, co] (host provides a pure layout permute),
    quantized on-chip to bf16.
  - The 3x3 conv is 9 shifted matmuls accumulated in PSUM: for each output
    row-chunk of 8 rows (N=448 <= 512 PSUM bank) and each 128-wide cout
    chunk, out[co, y, x] += qw[tap][ci, co].T @ qx[ci, y+dy, x+dx].
  - Epilogue fuses rescale + bias on ScalarE from PSUM, DMA out.
"""

import numpy as np

import concourse.bass as bass
import concourse.mybir as mybir
import concourse.tile as tile
from concourse import bacc
from concourse.bass_utils import run_bass_kernel_spmd

# Problem constants
B, CIN, COUT, H, W, KS = 32, 128, 256, 56, 56, 3
NCORES = 8
BPC = B // NCORES          # images per core
NPIX = H * W               # 3136
HP = H + 2                 # padded spatial
QL = 127.0
SX = QL / 3.0              # activation quant scale
SW = QL / 0.05             # weight quant scale
RESCALE = (3.0 * 0.05) / (QL * QL)
MAGIC = 1.5 * 2.0**23      # fp32 round-to-nearest-even trick

ROWS = 8                   # output rows per matmul tile
RT = H // ROWS             # 7 row tiles per image
NTAP = KS * KS
NCHUNK = COUT // 128       # 2 cout chunks
GROUP = 4                  # psum tiles sharing one weight load

F32 = mybir.dt.float32
BF16 = mybir.dt.bfloat16

_NC = None


def _build(reps: int = 1, no_in: bool = False, no_out: bool = False,
           no_mm: bool = False):
    """Build the SPMD graph. reps>1 wraps the whole per-call pipeline in a
    hardware For loop — used only by the timing harness (bench.py) to
    measure per-iteration HW time through the high-latency tunnel.
    no_in/no_out/no_mm ablate pipeline stages for bottleneck hunting."""
    nc = bacc.Bacc("TRN2", target_bir_lowering=False, num_devices=NCORES)

    x_t = nc.dram_tensor("x", [BPC, CIN, NPIX], F32, kind="ExternalInput")
    w_t = nc.dram_tensor("weight", [NTAP, CIN, COUT], F32, kind="ExternalInput")
    b_t = nc.dram_tensor("bias", [NCHUNK, 128, 1], F32, kind="ExternalInput")
    o_t = nc.dram_tensor("out", [BPC, NCHUNK, 128, NPIX], F32, kind="ExternalOutput")

    with tile.TileContext(nc) as tc:
        with (
            tc.tile_pool(name="consts", bufs=1) as consts,
            tc.tile_pool(name="xq", bufs=1) as xqp,
            tc.tile_pool(name="xstage", bufs=2) as xsp,
            tc.tile_pool(name="tmp", bufs=2) as tmpp,
            tc.tile_pool(name="outp", bufs=6) as outp,
            tc.tile_pool(name="psum", bufs=8, space="PSUM") as psp,
        ):
            # ---- weights: DMA [ci, tap, co], quantize to bf16 ----
            wraw = consts.tile([128, NTAP, COUT], F32, tag="wraw")
            nc.sync.dma_start(out=wraw[:], in_=w_t[:].rearrange("t p c -> p t c"))
            wtmp = consts.tile([128, NTAP, COUT], F32, tag="wtmp")
            nc.scalar.activation(
                wtmp[:], wraw[:], mybir.ActivationFunctionType.Copy,
                bias=MAGIC, scale=SW,
            )
            nc.vector.tensor_scalar(
                wtmp[:], wtmp[:], MAGIC, -QL,
                mybir.AluOpType.subtract, mybir.AluOpType.max,
            )
            wq = consts.tile([128, NTAP, COUT], BF16, tag="wq")
            nc.vector.tensor_scalar_min(wq[:], wtmp[:], QL)

            bias_sb = []
            for c in range(NCHUNK):
                bs = consts.tile([128, 1], F32, tag=f"bias{c}")
                nc.sync.dma_start(out=bs[:], in_=b_t[c])
                bias_sb.append(bs)

            # ---- padded quantized activations, one tile per image ----
            xq = []
            for b in range(BPC):
                t = xqp.tile([128, HP, HP], BF16, tag=f"xq{b}")
                nc.vector.memset(t[:], 0.0)
                xq.append(t)

            def body(_iv=None):
                for b in range(BPC) if not no_in else []:
                    xs = xsp.tile([128, NPIX], F32, tag="xs", name=f"xs{b}")
                    nc.sync.dma_start(out=xs[:], in_=x_t[b])
                    t1 = tmpp.tile([128, NPIX], F32, tag="t1", name=f"t1_{b}")
                    nc.scalar.activation(
                        t1[:], xs[:], mybir.ActivationFunctionType.Copy,
                        bias=MAGIC, scale=SX,
                    )
                    nc.vector.tensor_scalar(
                        t1[:], t1[:], MAGIC, -QL,
                        mybir.AluOpType.subtract, mybir.AluOpType.max,
                    )
                    nc.vector.tensor_scalar_min(
                        xq[b][:, 1 : H + 1, 1 : W + 1],
                        t1[:].rearrange("p (h w) -> p h w", h=H),
                        QL,
                    )

                # ---- conv: 9 shifted matmuls accumulated in PSUM ----
                tiles = [(b, r) for b in range(BPC) for r in range(RT)]
                assert len(tiles) % GROUP == 0
                for g in range(len(tiles) // GROUP):
                    grp = tiles[GROUP * g : GROUP * (g + 1)]
                    for c in range(NCHUNK):
                        pts = [
                            psp.tile([128, ROWS * W], F32, tag="pt",
                                     name=f"pt{g}_{c}_{i}")
                            for i, _ in enumerate(grp)
                        ]
                        for tap in range(NTAP):
                            ky, kx = divmod(tap, KS)
                            lhsT = wq[:, tap, c * 128 : (c + 1) * 128]
                            for t, (b, r) in enumerate(grp):
                                if no_mm:
                                    continue
                                rhs = xq[b][:, r * ROWS + ky : r * ROWS + ky + ROWS,
                                            kx : kx + W]
                                nc.tensor.matmul(
                                    pts[t][:], lhsT, rhs,
                                    start=(tap == 0), stop=(tap == NTAP - 1),
                                )
                        for t, (b, r) in enumerate(grp):
                            ot = outp.tile([128, ROWS * W], F32, tag="ot",
                                           name=f"ot{g}_{c}_{t}")
                            nc.scalar.activation(
                                ot[:], pts[t][:],
                                mybir.ActivationFunctionType.Identity,
                                bias=bias_sb[c][:], scale=RESCALE,
                            )
                            if not no_out:
                                nc.sync.dma_start(
                                    out=o_t[b, c, :, r * ROWS * W : (r + 1) * ROWS * W],
                                    in_=ot[:],
                                )

            if reps == 1:
                body()
            else:
                with tc.For_i(0, reps, 1):
                    body()
    nc.compile()
    return nc


def _get_nc():
    global _NC
    if _NC is None:
        _NC = _build()
    return _NC


def kernel(x: np.ndarray, weight: np.ndarray, bias: np.ndarray, trace: bool = False):
    """Full inputs in, full output out. Shards batch across 8 cores."""
    x = np.ascontiguousarray(x, dtype=np.float32).reshape(B, CIN, NPIX)
    # pure layout permute: [co, ci, ky, kx] -> [ky*kx, ci, co]
    w_l = np.ascontiguousarray(
        weight.astype(np.float32).transpose(2, 3, 1, 0)
    ).reshape(NTAP, CIN, COUT)
    b_l = np.ascontiguousarray(bias.astype(np.float32)).reshape(NCHUNK, 128, 1)

    nc = _get_nc()
    in_maps = [
        {
            "x": np.ascontiguousarray(x[i * BPC : (i + 1) * BPC]),
            "weight": w_l,
            "bias": b_l,
        }
        for i in range(NCORES)
    ]
    res = run_bass_kernel_spmd(nc, in_maps, core_ids=list(range(NCORES)), trace=trace)
    out = np.concatenate(
        [r["out"].reshape(BPC, COUT, H, W) for r in res.results], axis=0
    )
    if trace:
        kernel.last_results = res
    return out
